# revision 27
# baseline (speedup 1.0000x reference)
"""E3Conv Trainium2 kernel: 8-core SPMD, dst-partitioned edges.

Core i owns nodes [1250i,1250(i+1)) and all edges into them (no collective).
Host precomputes per-edge geometry features (radial basis, spherical-harmonic
planes) and per-edge node-scalar gathers (node MLP degenerates to a 10-row
atom-type table), laid out as [rows, E] bf16 planes streamed per tile.
Device per 896-edge tile: radial MLP (PE matmul + Act silu), quadrant
tensor-product contraction (PE, f32 PSUM), aiS/aiD multiplies (DVE/Pool),
L2 reduction, transpose, and dma_scatter_add straight into an HBM
accumulator. Edges are host-packed so each 512/384-token scatter span has
unique destinations (CCE-add races within one DMA, accumulates across DMAs).
Scatter-mean division by counts happens on host.
"""
import sys
sys.path.insert(0, "/opt/trn_rl_repo")
import numpy as np

import concourse.bass as bass
import concourse.tile as tile
from concourse import bacc, mybir
from concourse import bass_utils

P = 128
N_NODES, N_EDGES, N_GRAPHS = 10000, 131072, 64
N_CORES, NPC = 8, 1250
OUTR = 1280              # out rows per core (1250 real + junk row 1250)
JUNK = NPC               # scatter row for padded tokens
MAX_RADIUS, N_BASIS = 4.0, 10
STEP = MAX_RADIUS / (N_BASIS + 1)
VCENTERS = np.linspace(0.0, MAX_RADIUS, N_BASIS + 2)[1:-1].astype(np.float32)
F32, BF16, I16 = mybir.dt.float32, mybir.dt.bfloat16, mybir.dt.int16
AF = mybir.ActivationFunctionType
ALU = mybir.AluOpType
TILE_CH = 7
ET = TILE_CH * P         # 896 edges per tile
NSL = [(0, 512), (512, ET)]
SPAN_CH = (4, 3)         # scatter spans per tile: 512 + 384 tokens


def _silu(x):
    return x / (1.0 + np.exp(-x))


def _build_consts(fW3, fW4):
    """W4p [128,4*224] quadrant-stationary layout, Sel-fused fW3p [64,512],
    L2A/L2B [112,60] u-reduction matrices."""
    s3 = 3.0 ** 0.5
    W4p = np.zeros((512, 224), np.float32)
    offs = {0: 0, 1: 1024, 2: 1536}
    wbase = {0: 0, 1: 16, 2: 24}
    scale_l = {0: 1.0 / 64, 1: s3 / 64, 2: 1.0 / 64}
    for l, mo in enumerate((16, 8, 4)):
        for u in range(8):
            for v in range(8):
                for wl in range(mo):
                    col = offs[l] + (u * 8 + v) * mo + wl
                    w = wbase[l] + wl
                    W4p[np.arange(64) * 8 + v, w * 8 + u] = fW4[:, col] * scale_l[l]
    W4pt = np.ascontiguousarray(
        np.transpose(W4p.reshape(4, 128, 224), (1, 0, 2)).reshape(128, 896))
    Sel = np.zeros((4, 64, 128), np.float32)
    for q in range(4):
        for r in range(128):
            Sel[q, 16 * q + r // 8, r] = 1.0
    fW3p = np.ascontiguousarray(np.concatenate(
        [(fW3 / 8.0) @ Sel[q] for q in range(4)], axis=1))  # [64, 512]
    L2A = np.zeros((112, 60), np.float32)
    L2B = np.zeros((112, 60), np.float32)
    for r in range(112):
        L2A[r, r // 8] = 1.0
        w = 14 + r // 8
        if w < 16:
            L2B[r, w] = 1.0
        elif w < 24:
            for m in range(3):
                L2B[r, 16 + (w - 16) * 3 + m] = 1.0
        else:
            for k in range(5):
                L2B[r, 40 + (w - 24) * 5 + k] = 1.0
    return W4pt, fW3p, L2A, L2B


def _pack_spans(dstl, C_TOT):
    """Assign each edge to a scatter span so no span repeats a destination.
    Spans alternate capacity 512/384 (chunk-aligned halves of each tile).
    Returns token->edge map [E] (-1 = pad) or None if infeasible."""
    NT = C_TOT // TILE_CH
    nspan = 2 * NT
    caps = np.where(np.arange(nspan) % 2 == 0, SPAN_CH[0] * P, SPAN_CH[1] * P)
    load = np.zeros(nspan, np.int64)
    fill = [[] for _ in range(nspan)]
    order = np.argsort(dstl, kind="stable")
    bounds = np.searchsorted(dstl[order], np.arange(NPC + 1))
    groups = [(bounds[n + 1] - bounds[n], order[bounds[n]:bounds[n + 1]])
              for n in range(NPC)]
    groups.sort(key=lambda g: -g[0])
    for c, elist in groups:
        if c == 0:
            break
        if c > nspan:
            return None
        rem = caps - load
        sel = np.argpartition(-rem, c - 1)[:c]
        if rem[sel].min() <= 0:
            return None
        for s, e in zip(sel, elist):
            fill[s].append(e)
            load[s] += 1
    tok = np.full(C_TOT * P, -1, np.int64)
    for s in range(nspan):
        t, half = s // 2, s % 2
        start = t * ET + (0 if half == 0 else SPAN_CH[0] * P)
        tok[start:start + load[s]] = fill[s]
    return tok


def _host_prep(inputs):
    pos = np.asarray(inputs["pos"], np.float32)
    A = np.asarray(inputs["A"]).astype(np.int64)
    batch = np.asarray(inputs["batch"]).astype(np.int64)
    esrc = np.asarray(inputs["edge_src"]).astype(np.int64)
    edst = np.asarray(inputs["edge_dst"]).astype(np.int64)
    shifts = np.asarray(inputs["edge_shifts"], np.float32)
    cell = np.asarray(inputs["cell"], np.float32)
    counts = np.bincount(edst, minlength=N_NODES).astype(np.float32)
    cpn = cell[batch]                                   # [N,3,3]

    # node MLP is atom-type degenerate: 10-row table on host
    et = np.asarray(inputs["embed_table"], np.float32)
    h = _silu(et @ np.asarray(inputs["fit_W1"], np.float32)
              + np.asarray(inputs["fit_b1"], np.float32))
    h = _silu(h @ np.asarray(inputs["fit_W2"], np.float32)
              + np.asarray(inputs["fit_b2"], np.float32))
    AiTab = (h @ np.asarray(inputs["fit_W3"], np.float32)
             + np.asarray(inputs["fit_b3"], np.float32))  # [10, 8]
    AiA = AiTab[A]                                        # [N, 8]

    core_edges = []
    cmax = 0
    for ci in range(N_CORES):
        lo = ci * NPC
        ids = np.nonzero((edst >= lo) & (edst < lo + NPC))[0]
        core_edges.append(ids)
        cmax = max(cmax, (len(ids) + P - 1) // P)
    C_TOT = ((cmax + TILE_CH - 1) // TILE_CH) * TILE_CH
    toks = None
    while toks is None:
        toks = []
        for ci in range(N_CORES):
            t = _pack_spans(edst[core_edges[ci]] - ci * NPC, C_TOT)
            if t is None:
                toks = None
                C_TOT += TILE_CH
                break
            toks.append(t)
    E = C_TOT * P

    import ml_dtypes
    per_core = []
    for ci in range(N_CORES):
        ids = core_edges[ci]
        tok = toks[ci]                      # [E] -> index into ids, or -1
        pad = tok < 0
        e_ids = np.where(pad, 0, ids[np.maximum(tok, 0)])
        src = np.where(pad, 0, esrc[e_ids])
        dstg = np.where(pad, 0, edst[e_ids])
        dstl = np.where(pad, JUNK, dstg - ci * NPC).astype(np.int16)
        sh = np.where(pad[:, None], 0.0, shifts[e_ids]).astype(np.float32)
        sv = np.einsum('ei,eij->ej', sh, cpn[src])
        ev = pos[dstg] - pos[src] + sv
        L = np.sqrt((ev * ev).sum(1))
        u = ev / np.maximum(L, 1e-9)[:, None]
        x, y, z = u[:, 0], u[:, 1], u[:, 2]
        s5, s15 = 5.0 ** 0.5, 15.0 ** 0.5
        sh2 = np.stack([s15 * x * z, s15 * x * y,
                        s5 * (y * y - 0.5 * (x * x + z * z)),
                        s15 * y * z, 0.5 * s15 * (z * z - x * x)], -1)
        diff = (L[:, None] - VCENTERS) / STEP
        bas = np.exp(-diff * diff) / 1.12                # [E,10]
        shs = np.empty((60, E), np.float32)
        shs[0:16] = 1.0
        for w in range(8):
            shs[16 + 3 * w:19 + 3 * w] = u.T
        for w in range(4):
            shs[40 + 5 * w:45 + 5 * w] = sh2.T
        wr = dstl.reshape(-1, 16).T                      # [16, E/16]
        NT = C_TOT // TILE_CH
        geo = np.zeros((124, E), np.float32)             # bas@0, shs@64
        geo[0:10] = bas.T
        geo[64:124] = shs
        aiS = np.tile(AiA[src].T, (16, 1)).reshape(P, NT, ET)
        aiD = np.tile(AiA[dstg].T, (16, 1)).reshape(P, NT, ET)
        ai = np.stack([aiS, aiD], axis=2).reshape(P, 2 * E)
        per_core.append(dict(
            geo=np.ascontiguousarray(geo).astype(ml_dtypes.bfloat16),
            ai=np.ascontiguousarray(ai).astype(ml_dtypes.bfloat16),
            idx=np.ascontiguousarray(np.tile(wr, (8, 1))),
        ))
    return per_core, counts, C_TOT


def _build_bass(C_TOT):
    NT = C_TOT // TILE_CH
    E = C_TOT * P
    nc = bacc.Bacc("TRN2", target_bir_lowering=False, debug=False,
                   num_devices=N_CORES)

    def din(name, shape, dt=BF16):
        return nc.dram_tensor(name, shape, dt, kind="ExternalInput").ap()

    geo_d = din("geo", [124, E])
    ai_d = din("ai", [P, 2 * E])
    idx_d = din("idx", [P, E // 16], I16)
    blob_d = din("blob", [P, 1716])
    mini_d = din("mini", [64, 128])
    out_d = nc.dram_tensor("out", [OUTR, 64], F32, kind="ExternalOutput").ap()

    with tile.TileContext(nc) as tc:
        with tc.tile_pool(name="const", bufs=1) as cp, \
             tc.tile_pool(name="sb", bufs=2) as sp, \
             tc.tile_pool(name="inp", bufs=3) as ip, \
             tc.tile_pool(name="big", bufs=2, space="PSUM") as pb, \
             tc.tile_pool(name="pc", bufs=1, space="PSUM") as pc, \
             tc.tile_pool(name="pt", bufs=1, space="PSUM") as pt:
            mini = cp.tile([64, 128], BF16, tag="mini")
            nc.sync.dma_start(mini[:], mini_d[:])
            fW2p = mini[0:64, 0:64]
            fW1p = mini[0:10, 64:128]
            blob = cp.tile([P, 1716], BF16, tag="blob")
            identb = blob[0:60, 1656:1716]
            fW3p = blob[0:64, 64:576]
            W4pt = blob[:, 576:1472]
            L2At = blob[0:112, 1472:1532]
            L2Bt = blob[0:112, 1532:1592]
            idx = cp.tile([P, E // 16], I16, tag="idx")
            zsb = cp.tile([P, OUTR * 64 // P], F32, tag="zsb")

            def prologue_rest():
                # deferred: blob / idx / HBM-accumulator zeroing issue after
                # the first tile's input DMAs so compute starts immediately
                nc.sync.dma_start(blob[:], blob_d[:])
                nc.sync.dma_start(idx[:], idx_d[:])
                nc.gpsimd.memset(zsb[:], 0.0)
                nc.sync.dma_start(
                    out_d[:].rearrange("(c p) e -> p c e", p=P),
                    zsb[:].rearrange("p (c e) -> p c e", e=64))

            def stF_dma(t):
                geo = ip.tile([124, ET], BF16, tag="geo", bufs=5)
                nc.sync.dma_start(geo[:], geo_d[:, t * ET:(t + 1) * ET])
                ai = ip.tile([P, 2 * ET], BF16, tag="ai", bufs=4)
                nc.sync.dma_start(ai[:], ai_d[:, 2 * t * ET:2 * (t + 1) * ET])
                return dict(t=t, bas=geo[0:10, :], shs=geo[64:124, :],
                            aiS=ai[:, 0:ET], aiD=ai[:, ET:2 * ET])

            def stF_h1(cur):
                h1p = pb.tile([P, ET], F32, tag="big", name="h1p")
                for a, b in NSL:
                    nc.tensor.matmul(h1p[0:64, a:b], fW1p,
                                     cur["bas"][:, a:b], start=True, stop=True)
                h1 = sp.tile([64, ET], BF16, tag="h1")
                nc.scalar.activation(h1[:], h1p[0:64, :], AF.Silu)
                cur["h1"] = h1

            def stF_h2(cur):
                h2p = pb.tile([P, ET], F32, tag="big", name="h2p")
                for a, b in NSL:
                    nc.tensor.matmul(h2p[0:64, a:b], fW2p,
                                     cur["h1"][:, a:b], start=True, stop=True)
                h2 = sp.tile([64, ET], BF16, tag="h2")
                nc.scalar.activation(h2[:], h2p[0:64, :], AF.Silu)
                cur["h2"] = h2

            POOLQ = (1,)           # this quadrant's rq runs on gpsimd
            QORD = (0, 2, 3, 1)    # cps pass order: pool quadrant last

            def q_wrp(cur, q):
                wrp = pb.tile([P, ET], F32, tag="big", name=f"wrp{q}")
                for a, b in NSL:
                    nc.tensor.matmul(wrp[:, a:b],
                                     fW3p[:, q * 128:(q + 1) * 128],
                                     cur["h2"][:, a:b], start=True, stop=True)
                wS = sp.tile([P, ET], BF16, tag=f"wS{q}")
                nc.scalar.activation(wS[:], wrp[:], AF.Silu)
                cur.setdefault("wS", {})[q] = wS

            def q_rq(cur, q):
                rq = sp.tile([P, ET], BF16, tag=f"rq{q}")
                eng = nc.gpsimd if q in POOLQ else nc.vector
                eng.tensor_tensor(out=rq[:], in0=cur["wS"][q][:],
                                  in1=cur["aiD"][:], op=ALU.mult)
                cur.setdefault("rqs", {})[q] = rq

            CT = {0: "c01", 1: "c23"}

            def cps_open(cur, m):
                cur[f"ch{m}"] = pc.tile([112, ET], F32, tag=CT[m],
                                        name=f"cp{m}")

            def cps_pass(cur, m, qi):
                q = QORD[qi]
                for a, b in NSL:
                    nc.tensor.matmul(
                        cur[f"ch{m}"][:, a:b],
                        W4pt[:, q * 224 + m * 112:q * 224 + (m + 1) * 112],
                        cur["rqs"][q][:, a:b], start=(qi == 0), stop=(qi == 3))

            def cps_close(cur, m):
                tm = sp.tile([112, ET], BF16, tag=f"tm{m}")
                nc.vector.tensor_tensor(out=tm[:], in0=cur[f"ch{m}"][:],
                                        in1=cur["aiS"][0:112, :], op=ALU.mult)
                cur.setdefault("tms", []).append(tm)

            def stB1(cur):
                tms = cur["tms"]
                fps = pc.tile([112, ET], F32, tag="c01", name="fps")
                for a, b in NSL:
                    nc.tensor.matmul(fps[0:60, a:b], L2At,
                                     tms[0][:, a:b], start=True, stop=False)
                    nc.tensor.matmul(fps[0:60, a:b], L2Bt,
                                     tms[1][:, a:b], start=False, stop=True)
                cur["fps"] = fps

            def stB2a(cur):
                fps, shs = cur["fps"], cur["shs"]
                F = sp.tile([60, ET], BF16, tag="F")
                nc.vector.tensor_tensor(out=F[:], in0=fps[0:60, :],
                                        in1=shs[:, :], op=ALU.mult)
                cur["F"] = F

            def stB2t(cur):
                F = cur["F"]
                ftp = pc.tile([P, TILE_CH * 60], BF16, tag="c23", name="ftp")
                for cc in range(TILE_CH):
                    nc.tensor.transpose(ftp[:, cc * 60:(cc + 1) * 60],
                                        F[:, cc * P:(cc + 1) * P],
                                        identb)
                cur["ftp"] = ftp

            def stB2s(cur):
                t, ftp = cur["t"], cur["ftp"]
                fc = sp.tile([P, TILE_CH * 64], F32, tag="fc")
                fc3 = fc[:].rearrange("p (c e) -> p c e", e=64)
                nc.vector.tensor_copy(
                    fc3[:, :, 0:60],
                    ftp[:].rearrange("p (c e) -> p c e", e=60))
                c0 = 0
                for s, sch in enumerate(SPAN_CH):
                    n = sch * P
                    icol = t * (ET // 16) + c0 * P // 16
                    nc.gpsimd.dma_scatter_add(
                        out_d[:], fc3[:, c0:c0 + sch, :],
                        idx[:, icol:icol + n // 16], n, n, 64)
                    c0 += sch

            # 4-deep software pipeline; tiles in flight per iteration i:
            # F(i) Q(i-1) M(i-2) B(i-3).  Act order per iteration is
            # h1(i), q0(i-1), h2(i), q1, q2, q3 -- each silu's input matmul
            # is issued one Act-slot ahead so the silu chain runs gapless;
            # cps passes fill PE gaps; fps(i-2) runs at end of iteration so
            # Fmul(i-3) fires at the start of the next one.
            tiles = {}
            for i in range(NT + 3):
                f = tiles.setdefault(i, stF_dma(i)) if i < NT else None
                if i == 0:
                    prologue_rest()
                q, m, b = tiles.get(i - 1), tiles.get(i - 2), tiles.get(i - 3)
                if f:
                    stF_h1(f)
                if b:
                    stB1(b)          # fps right after h1p (waits tm1(i-3))
                if q:
                    q_wrp(q, 0)
                if b:
                    stB2a(b)         # Fmul at DVE head
                if f:
                    stF_h2(f)
                if m:
                    cps_open(m, 0)
                    cps_pass(m, 0, 0)
                    cps_pass(m, 0, 1)
                if q:
                    q_wrp(q, 1)
                    q_rq(q, 0)
                if b:
                    stB2t(b)         # transposes (Fmul done by now)
                    stB2s(b)         # fc + scatters immediately after
                    del tiles[i - 3]
                if m:
                    cps_pass(m, 0, 2)
                    cps_pass(m, 0, 3)
                    cps_close(m, 0)
                if q:
                    q_wrp(q, 2)
                    q_rq(q, 1)
                if m:
                    cps_open(m, 1)
                    cps_pass(m, 1, 0)
                    cps_pass(m, 1, 1)
                if q:
                    q_rq(q, 2)
                    q_wrp(q, 3)
                if m:
                    cps_pass(m, 1, 2)
                    cps_pass(m, 1, 3)
                    cps_close(m, 1)
                if q:
                    q_rq(q, 3)
    nc.compile()
    return nc


_CACHE = {}


def kernel(**inputs):
    per_core, counts, C_TOT = _host_prep(inputs)
    W4pt, fW3p, L2A, L2B = _build_consts(
        np.asarray(inputs["fc_W3"], np.float32),
        np.asarray(inputs["fc_W4"], np.float32))
    if C_TOT not in _CACHE:
        _CACHE[C_TOT] = _build_bass(C_TOT)
    nc = _CACHE[C_TOT]
    import ml_dtypes
    blob = np.zeros((P, 1716), np.float32)
    blob[0:60, 1656:1716] = np.eye(60, dtype=np.float32)
    blob[0:64, 0:64] = np.asarray(inputs["fc_W2"], np.float32) / 8.0
    blob[0:64, 64:576] = fW3p
    blob[:, 576:1472] = W4pt
    blob[0:112, 1472:1532] = L2A
    blob[0:112, 1532:1592] = L2B
    blob[0:10, 1592:1656] = np.asarray(inputs["fc_W1"], np.float32)
    mini = np.zeros((64, 128), np.float32)
    mini[0:64, 0:64] = blob[0:64, 0:64]
    mini[0:10, 64:128] = np.asarray(inputs["fc_W1"], np.float32)
    shared = dict(blob=blob.astype(ml_dtypes.bfloat16),
                  mini=mini.astype(ml_dtypes.bfloat16))
    in_maps = []
    for ci in range(N_CORES):
        m = dict(shared)
        m.update(per_core[ci])
        in_maps.append(m)
    res = bass_utils.run_bass_kernel_spmd(nc, in_maps,
                                          core_ids=list(range(N_CORES)))
    out = np.concatenate([res.results[ci]["out"][:NPC, :60]
                          for ci in range(N_CORES)], 0)
    return (out / np.maximum(counts, 1.0)[:, None]).astype(np.float32)


# revision 28
# speedup vs baseline: 1.0003x; 1.0003x over previous
"""E3Conv Trainium2 kernel: 8-core SPMD, dst-partitioned edges.

Core i owns nodes [1250i,1250(i+1)) and all edges into them (no collective).
Host precomputes per-edge geometry features (radial basis, spherical-harmonic
planes) and per-edge node-scalar gathers (node MLP degenerates to a 10-row
atom-type table), laid out as [rows, E] bf16 planes streamed per tile.
Device per 896-edge tile: radial MLP (PE matmul + Act silu), quadrant
tensor-product contraction (PE, f32 PSUM), aiS/aiD multiplies (DVE/Pool),
L2 reduction, transpose, and dma_scatter_add straight into an HBM
accumulator. Edges are host-packed so each 512/384-token scatter span has
unique destinations (CCE-add races within one DMA, accumulates across DMAs).
Scatter-mean division by counts happens on host.
"""
import sys
sys.path.insert(0, "/opt/trn_rl_repo")
import numpy as np

import concourse.bass as bass
import concourse.tile as tile
from concourse import bacc, mybir
from concourse import bass_utils
from concourse.masks import make_identity

P = 128
N_NODES, N_EDGES, N_GRAPHS = 10000, 131072, 64
N_CORES, NPC = 8, 1250
OUTR = 1280              # out rows per core (1250 real + junk row 1250)
JUNK = NPC               # scatter row for padded tokens
MAX_RADIUS, N_BASIS = 4.0, 10
STEP = MAX_RADIUS / (N_BASIS + 1)
VCENTERS = np.linspace(0.0, MAX_RADIUS, N_BASIS + 2)[1:-1].astype(np.float32)
F32, BF16, I16 = mybir.dt.float32, mybir.dt.bfloat16, mybir.dt.int16
AF = mybir.ActivationFunctionType
ALU = mybir.AluOpType
TILE_CH = 7
ET = TILE_CH * P         # 896 edges per tile
NSL = [(0, 512), (512, ET)]
SPAN_CH = (4, 3)         # scatter spans per tile: 512 + 384 tokens


def _silu(x):
    return x / (1.0 + np.exp(-x))


def _build_consts(fW3, fW4):
    """W4p [128,4*224] quadrant-stationary layout, Sel-fused fW3p [64,512],
    L2A/L2B [112,60] u-reduction matrices."""
    s3 = 3.0 ** 0.5
    W4p = np.zeros((512, 224), np.float32)
    offs = {0: 0, 1: 1024, 2: 1536}
    wbase = {0: 0, 1: 16, 2: 24}
    scale_l = {0: 1.0 / 64, 1: s3 / 64, 2: 1.0 / 64}
    for l, mo in enumerate((16, 8, 4)):
        for u in range(8):
            for v in range(8):
                for wl in range(mo):
                    col = offs[l] + (u * 8 + v) * mo + wl
                    w = wbase[l] + wl
                    W4p[np.arange(64) * 8 + v, w * 8 + u] = fW4[:, col] * scale_l[l]
    W4pt = np.ascontiguousarray(
        np.transpose(W4p.reshape(4, 128, 224), (1, 0, 2)).reshape(128, 896))
    Sel = np.zeros((4, 64, 128), np.float32)
    for q in range(4):
        for r in range(128):
            Sel[q, 16 * q + r // 8, r] = 1.0
    fW3p = np.ascontiguousarray(np.concatenate(
        [(fW3 / 8.0) @ Sel[q] for q in range(4)], axis=1))  # [64, 512]
    L2A = np.zeros((112, 60), np.float32)
    L2B = np.zeros((112, 60), np.float32)
    for r in range(112):
        L2A[r, r // 8] = 1.0
        w = 14 + r // 8
        if w < 16:
            L2B[r, w] = 1.0
        elif w < 24:
            for m in range(3):
                L2B[r, 16 + (w - 16) * 3 + m] = 1.0
        else:
            for k in range(5):
                L2B[r, 40 + (w - 24) * 5 + k] = 1.0
    return W4pt, fW3p, L2A, L2B


def _pack_spans(dstl, C_TOT):
    """Assign each edge to a scatter span so no span repeats a destination.
    Spans alternate capacity 512/384 (chunk-aligned halves of each tile).
    Returns token->edge map [E] (-1 = pad) or None if infeasible."""
    NT = C_TOT // TILE_CH
    nspan = 2 * NT
    caps = np.where(np.arange(nspan) % 2 == 0, SPAN_CH[0] * P, SPAN_CH[1] * P)
    load = np.zeros(nspan, np.int64)
    fill = [[] for _ in range(nspan)]
    order = np.argsort(dstl, kind="stable")
    bounds = np.searchsorted(dstl[order], np.arange(NPC + 1))
    groups = [(bounds[n + 1] - bounds[n], order[bounds[n]:bounds[n + 1]])
              for n in range(NPC)]
    groups.sort(key=lambda g: -g[0])
    for c, elist in groups:
        if c == 0:
            break
        if c > nspan:
            return None
        rem = caps - load
        sel = np.argpartition(-rem, c - 1)[:c]
        if rem[sel].min() <= 0:
            return None
        for s, e in zip(sel, elist):
            fill[s].append(e)
            load[s] += 1
    tok = np.full(C_TOT * P, -1, np.int64)
    for s in range(nspan):
        t, half = s // 2, s % 2
        start = t * ET + (0 if half == 0 else SPAN_CH[0] * P)
        tok[start:start + load[s]] = fill[s]
    return tok


def _host_prep(inputs):
    pos = np.asarray(inputs["pos"], np.float32)
    A = np.asarray(inputs["A"]).astype(np.int64)
    batch = np.asarray(inputs["batch"]).astype(np.int64)
    esrc = np.asarray(inputs["edge_src"]).astype(np.int64)
    edst = np.asarray(inputs["edge_dst"]).astype(np.int64)
    shifts = np.asarray(inputs["edge_shifts"], np.float32)
    cell = np.asarray(inputs["cell"], np.float32)
    counts = np.bincount(edst, minlength=N_NODES).astype(np.float32)
    cpn = cell[batch]                                   # [N,3,3]

    # node MLP is atom-type degenerate: 10-row table on host
    et = np.asarray(inputs["embed_table"], np.float32)
    h = _silu(et @ np.asarray(inputs["fit_W1"], np.float32)
              + np.asarray(inputs["fit_b1"], np.float32))
    h = _silu(h @ np.asarray(inputs["fit_W2"], np.float32)
              + np.asarray(inputs["fit_b2"], np.float32))
    AiTab = (h @ np.asarray(inputs["fit_W3"], np.float32)
             + np.asarray(inputs["fit_b3"], np.float32))  # [10, 8]
    AiA = AiTab[A]                                        # [N, 8]

    core_edges = []
    cmax = 0
    for ci in range(N_CORES):
        lo = ci * NPC
        ids = np.nonzero((edst >= lo) & (edst < lo + NPC))[0]
        core_edges.append(ids)
        cmax = max(cmax, (len(ids) + P - 1) // P)
    C_TOT = ((cmax + TILE_CH - 1) // TILE_CH) * TILE_CH
    toks = None
    while toks is None:
        toks = []
        for ci in range(N_CORES):
            t = _pack_spans(edst[core_edges[ci]] - ci * NPC, C_TOT)
            if t is None:
                toks = None
                C_TOT += TILE_CH
                break
            toks.append(t)
    E = C_TOT * P

    import ml_dtypes
    per_core = []
    for ci in range(N_CORES):
        ids = core_edges[ci]
        tok = toks[ci]                      # [E] -> index into ids, or -1
        pad = tok < 0
        e_ids = np.where(pad, 0, ids[np.maximum(tok, 0)])
        src = np.where(pad, 0, esrc[e_ids])
        dstg = np.where(pad, 0, edst[e_ids])
        dstl = np.where(pad, JUNK, dstg - ci * NPC).astype(np.int16)
        sh = np.where(pad[:, None], 0.0, shifts[e_ids]).astype(np.float32)
        sv = np.einsum('ei,eij->ej', sh, cpn[src])
        ev = pos[dstg] - pos[src] + sv
        L = np.sqrt((ev * ev).sum(1))
        u = ev / np.maximum(L, 1e-9)[:, None]
        x, y, z = u[:, 0], u[:, 1], u[:, 2]
        s5, s15 = 5.0 ** 0.5, 15.0 ** 0.5
        sh2 = np.stack([s15 * x * z, s15 * x * y,
                        s5 * (y * y - 0.5 * (x * x + z * z)),
                        s15 * y * z, 0.5 * s15 * (z * z - x * x)], -1)
        diff = (L[:, None] - VCENTERS) / STEP
        bas = np.exp(-diff * diff) / 1.12                # [E,10]
        shs = np.empty((60, E), np.float32)
        shs[0:16] = 1.0
        for w in range(8):
            shs[16 + 3 * w:19 + 3 * w] = u.T
        for w in range(4):
            shs[40 + 5 * w:45 + 5 * w] = sh2.T
        wr = dstl.reshape(-1, 16).T                      # [16, E/16]
        NT = C_TOT // TILE_CH
        geo = np.zeros((124, E), np.float32)             # bas@0, shs@64
        geo[0:10] = bas.T
        geo[64:124] = shs
        aiS = np.tile(AiA[src].T, (16, 1)).reshape(P, NT, ET)
        aiD = np.tile(AiA[dstg].T, (16, 1)).reshape(P, NT, ET)
        ai = np.stack([aiS, aiD], axis=2).reshape(P, 2 * E)
        per_core.append(dict(
            geo=np.ascontiguousarray(geo).astype(ml_dtypes.bfloat16),
            ai=np.ascontiguousarray(ai).astype(ml_dtypes.bfloat16),
            idx=np.ascontiguousarray(np.tile(wr, (8, 1))),
        ))
    return per_core, counts, C_TOT


def _build_bass(C_TOT):
    NT = C_TOT // TILE_CH
    E = C_TOT * P
    nc = bacc.Bacc("TRN2", target_bir_lowering=False, debug=False,
                   num_devices=N_CORES)

    def din(name, shape, dt=BF16):
        return nc.dram_tensor(name, shape, dt, kind="ExternalInput").ap()

    geo_d = din("geo", [124, E])
    ai_d = din("ai", [P, 2 * E])
    idx_d = din("idx", [P, E // 16], I16)
    blob_d = din("blob", [P, 1656])
    mini_d = din("mini", [64, 128])
    out_d = nc.dram_tensor("out", [OUTR, 64], F32, kind="ExternalOutput").ap()

    with tile.TileContext(nc) as tc:
        with tc.tile_pool(name="const", bufs=1) as cp, \
             tc.tile_pool(name="sb", bufs=2) as sp, \
             tc.tile_pool(name="inp", bufs=3) as ip, \
             tc.tile_pool(name="big", bufs=2, space="PSUM") as pb, \
             tc.tile_pool(name="pc", bufs=1, space="PSUM") as pc, \
             tc.tile_pool(name="pt", bufs=1, space="PSUM") as pt:
            ident = cp.tile([P, P], F32)
            make_identity(nc, ident[:])
            identb = cp.tile([P, P], BF16)
            nc.vector.tensor_copy(identb[:], ident[:])

            mini = cp.tile([64, 128], BF16, tag="mini")
            nc.sync.dma_start(mini[:], mini_d[:])
            fW2p = mini[0:64, 0:64]
            fW1p = mini[0:10, 64:128]
            blob = cp.tile([P, 1656], BF16, tag="blob")
            fW3p = blob[0:64, 64:576]
            W4pt = blob[:, 576:1472]
            L2At = blob[0:112, 1472:1532]
            L2Bt = blob[0:112, 1532:1592]
            idx = cp.tile([P, E // 16], I16, tag="idx")
            zsb = cp.tile([P, OUTR * 64 // P], F32, tag="zsb")

            def prologue_rest():
                # deferred: blob / idx / HBM-accumulator zeroing issue after
                # the first tile's input DMAs so compute starts immediately
                nc.sync.dma_start(blob[:], blob_d[:])
                nc.sync.dma_start(idx[:], idx_d[:])
                nc.gpsimd.memset(zsb[:], 0.0)
                nc.sync.dma_start(
                    out_d[:].rearrange("(c p) e -> p c e", p=P),
                    zsb[:].rearrange("p (c e) -> p c e", e=64))

            def stF_dma(t):
                geo = ip.tile([124, ET], BF16, tag="geo", bufs=5)
                nc.sync.dma_start(geo[:], geo_d[:, t * ET:(t + 1) * ET])
                ai = ip.tile([P, 2 * ET], BF16, tag="ai", bufs=4)
                nc.sync.dma_start(ai[:], ai_d[:, 2 * t * ET:2 * (t + 1) * ET])
                return dict(t=t, bas=geo[0:10, :], shs=geo[64:124, :],
                            aiS=ai[:, 0:ET], aiD=ai[:, ET:2 * ET])

            def stF_h1(cur):
                h1p = pb.tile([P, ET], F32, tag="big", name="h1p")
                for a, b in NSL:
                    nc.tensor.matmul(h1p[0:64, a:b], fW1p,
                                     cur["bas"][:, a:b], start=True, stop=True)
                h1 = sp.tile([64, ET], BF16, tag="h1")
                nc.scalar.activation(h1[:], h1p[0:64, :], AF.Silu)
                cur["h1"] = h1

            def stF_h2(cur):
                h2p = pb.tile([P, ET], F32, tag="big", name="h2p")
                for a, b in NSL:
                    nc.tensor.matmul(h2p[0:64, a:b], fW2p,
                                     cur["h1"][:, a:b], start=True, stop=True)
                h2 = sp.tile([64, ET], BF16, tag="h2")
                nc.scalar.activation(h2[:], h2p[0:64, :], AF.Silu)
                cur["h2"] = h2

            POOLQ = (1,)           # this quadrant's rq runs on gpsimd
            QORD = (0, 2, 3, 1)    # cps pass order: pool quadrant last

            def q_wrp(cur, q):
                wrp = pb.tile([P, ET], F32, tag="big", name=f"wrp{q}")
                for a, b in NSL:
                    nc.tensor.matmul(wrp[:, a:b],
                                     fW3p[:, q * 128:(q + 1) * 128],
                                     cur["h2"][:, a:b], start=True, stop=True)
                wS = sp.tile([P, ET], BF16, tag=f"wS{q}")
                nc.scalar.activation(wS[:], wrp[:], AF.Silu)
                cur.setdefault("wS", {})[q] = wS

            def q_rq(cur, q):
                rq = sp.tile([P, ET], BF16, tag=f"rq{q}")
                eng = nc.gpsimd if q in POOLQ else nc.vector
                eng.tensor_tensor(out=rq[:], in0=cur["wS"][q][:],
                                  in1=cur["aiD"][:], op=ALU.mult)
                cur.setdefault("rqs", {})[q] = rq

            CT = {0: "c01", 1: "c23"}

            def cps_open(cur, m):
                cur[f"ch{m}"] = pc.tile([112, ET], F32, tag=CT[m],
                                        name=f"cp{m}")

            def cps_pass(cur, m, qi):
                q = QORD[qi]
                for a, b in NSL:
                    nc.tensor.matmul(
                        cur[f"ch{m}"][:, a:b],
                        W4pt[:, q * 224 + m * 112:q * 224 + (m + 1) * 112],
                        cur["rqs"][q][:, a:b], start=(qi == 0), stop=(qi == 3))

            def cps_close(cur, m):
                tm = sp.tile([112, ET], BF16, tag=f"tm{m}")
                nc.vector.tensor_tensor(out=tm[:], in0=cur[f"ch{m}"][:],
                                        in1=cur["aiS"][0:112, :], op=ALU.mult)
                cur.setdefault("tms", []).append(tm)

            def stB1(cur):
                tms = cur["tms"]
                fps = pc.tile([112, ET], F32, tag="c01", name="fps")
                for a, b in NSL:
                    nc.tensor.matmul(fps[0:60, a:b], L2At,
                                     tms[0][:, a:b], start=True, stop=False)
                    nc.tensor.matmul(fps[0:60, a:b], L2Bt,
                                     tms[1][:, a:b], start=False, stop=True)
                cur["fps"] = fps

            def stB2a(cur):
                fps, shs = cur["fps"], cur["shs"]
                F = sp.tile([60, ET], BF16, tag="F")
                nc.vector.tensor_tensor(out=F[:], in0=fps[0:60, :],
                                        in1=shs[:, :], op=ALU.mult)
                cur["F"] = F

            def stB2t(cur):
                F = cur["F"]
                ftp = pc.tile([P, TILE_CH * 60], BF16, tag="c23", name="ftp")
                for cc in range(TILE_CH):
                    nc.tensor.transpose(ftp[:, cc * 60:(cc + 1) * 60],
                                        F[:, cc * P:(cc + 1) * P],
                                        identb[0:60, 0:60])
                cur["ftp"] = ftp

            def stB2s(cur):
                t, ftp = cur["t"], cur["ftp"]
                fc = sp.tile([P, TILE_CH * 64], F32, tag="fc")
                fc3 = fc[:].rearrange("p (c e) -> p c e", e=64)
                nc.vector.tensor_copy(
                    fc3[:, :, 0:60],
                    ftp[:].rearrange("p (c e) -> p c e", e=60))
                c0 = 0
                for s, sch in enumerate(SPAN_CH):
                    n = sch * P
                    icol = t * (ET // 16) + c0 * P // 16
                    nc.gpsimd.dma_scatter_add(
                        out_d[:], fc3[:, c0:c0 + sch, :],
                        idx[:, icol:icol + n // 16], n, n, 64)
                    c0 += sch

            # 4-deep software pipeline; tiles in flight per iteration i:
            # F(i) Q(i-1) M(i-2) B(i-3).  Act order per iteration is
            # h1(i), q0(i-1), h2(i), q1, q2, q3 -- each silu's input matmul
            # is issued one Act-slot ahead so the silu chain runs gapless;
            # cps passes fill PE gaps; fps(i-2) runs at end of iteration so
            # Fmul(i-3) fires at the start of the next one.
            tiles = {}
            for i in range(NT + 3):
                f = tiles.setdefault(i, stF_dma(i)) if i < NT else None
                if i == 0:
                    prologue_rest()
                q, m, b = tiles.get(i - 1), tiles.get(i - 2), tiles.get(i - 3)
                if f:
                    stF_h1(f)
                if b:
                    stB1(b)          # fps right after h1p (waits tm1(i-3))
                if q:
                    q_wrp(q, 0)
                if b:
                    stB2a(b)         # Fmul at DVE head
                if f:
                    stF_h2(f)
                if m:
                    cps_open(m, 0)
                    cps_pass(m, 0, 0)
                    cps_pass(m, 0, 1)
                if q:
                    q_wrp(q, 1)
                    q_rq(q, 0)
                if b:
                    stB2t(b)         # transposes (Fmul done by now)
                    stB2s(b)         # fc + scatters immediately after
                    del tiles[i - 3]
                if m:
                    cps_pass(m, 0, 2)
                    cps_pass(m, 0, 3)
                    cps_close(m, 0)
                if q:
                    q_wrp(q, 2)
                    q_rq(q, 1)
                if m:
                    cps_open(m, 1)
                    cps_pass(m, 1, 0)
                    cps_pass(m, 1, 1)
                if q:
                    q_rq(q, 2)
                    q_wrp(q, 3)
                if m:
                    cps_pass(m, 1, 2)
                    cps_pass(m, 1, 3)
                    cps_close(m, 1)
                if q:
                    q_rq(q, 3)
    nc.compile()
    return nc


_CACHE = {}


def kernel(**inputs):
    per_core, counts, C_TOT = _host_prep(inputs)
    W4pt, fW3p, L2A, L2B = _build_consts(
        np.asarray(inputs["fc_W3"], np.float32),
        np.asarray(inputs["fc_W4"], np.float32))
    if C_TOT not in _CACHE:
        _CACHE[C_TOT] = _build_bass(C_TOT)
    nc = _CACHE[C_TOT]
    import ml_dtypes
    blob = np.zeros((P, 1656), np.float32)
    blob[0:64, 0:64] = np.asarray(inputs["fc_W2"], np.float32) / 8.0
    blob[0:64, 64:576] = fW3p
    blob[:, 576:1472] = W4pt
    blob[0:112, 1472:1532] = L2A
    blob[0:112, 1532:1592] = L2B
    blob[0:10, 1592:1656] = np.asarray(inputs["fc_W1"], np.float32)
    mini = np.zeros((64, 128), np.float32)
    mini[0:64, 0:64] = blob[0:64, 0:64]
    mini[0:10, 64:128] = np.asarray(inputs["fc_W1"], np.float32)
    shared = dict(blob=blob.astype(ml_dtypes.bfloat16),
                  mini=mini.astype(ml_dtypes.bfloat16))
    in_maps = []
    for ci in range(N_CORES):
        m = dict(shared)
        m.update(per_core[ci])
        in_maps.append(m)
    res = bass_utils.run_bass_kernel_spmd(nc, in_maps,
                                          core_ids=list(range(N_CORES)))
    out = np.concatenate([res.results[ci]["out"][:NPC, :60]
                          for ci in range(N_CORES)], 0)
    return (out / np.maximum(counts, 1.0)[:, None]).astype(np.float32)
